# revision 81
# baseline (speedup 1.0000x reference)
"""Multi-head causal attention (B=2, T=2048, E=1024, H=16, D=64) on 8 trn2 cores.

Sharding: core c -> batch b = c // 4, head-group hg = c % 4 (4 heads each).
Per-core software pipeline, one iteration per 512-token chunk t:
  K proj (bf16) -> Q proj (fp8e4 DoubleRow over host-quantized x8/wq8) ->
  S^T[k,q] for q-block t (fp8e4 DoubleRow: Q^T/K^T are quantized into a
  d-half-split layout during PSUM evacuation, 0.5 cyc/row) -> exp stream on
  the act engine -> V proj (bf16) -> P@V + normalize for q-block t-1
  (deferred one iteration so block t's S keeps the act engine saturated) ->
  output projection for q-block t-2, interleaved as stall fillers.
Softmax denominator comes from a ones-augmented V matmul; P@V and the
row-parallel output projection run in bf16; the partial [T, E] leaves in
bf16 and the host sums the 4 partials per batch and adds the bias.
PE and act engines are co-critical (~91/~74 us busy of ~108 us total).
"""
import sys
from contextlib import ExitStack

sys.path.insert(0, "/opt/trn_rl_repo")

import numpy as np
import ml_dtypes

import concourse.bass as bass
import concourse.tile as tile
from concourse import bacc, mybir
from concourse.bass_utils import run_bass_kernel_spmd

F32 = mybir.dt.float32
F32R = mybir.dt.float32r
BF = mybir.dt.bfloat16
F8 = mybir.dt.float8e4
BF_NP = ml_dtypes.bfloat16
EXP = mybir.ActivationFunctionType.Exp

B, T, E, H = 2, 2048, 1024, 16
D = E // H              # 64
N_CORES = 8
GH = 4                  # heads per core
GE = GH * D             # 256 per-core projection width
SCALE = float(D) ** -0.5

TCH = 512               # projection t-chunk == attention q-block
NTCH = T // TCH         # 4
KC = 8                  # contraction chunks of 128 over E
QB = 512
NQB = T // QB           # 4
KB = 128                # attention k-block


DEFAULT_OPTS = dict(
    pv_stream=False,    # True: P@V streams V (65-row matmuls per q-tile);
                        # False: P@V streams P (baseline o^T layout)
    proj_bf16=True,     # projection matmul group dtype (x, wq, wk, wv)
    attn_bf16=True,     # attention matmul group dtype (qt, kt, P, v, wp)
    defer_outproj=True,  # emit qb's out-projection after the next chunk's
                         # projections so proj matmuls cover normalize latency
    norm_splits_last=1,
    dma_spread=True,    # issue startup DMAs across SP/DVE/Pool queues
    act_preload=True,   # dummy exp so the act-table load happens at t=0
    s_bufs=2,
    o_bufs=2,
    aux_bufs=2,
    p_bufs=40,
    x_bufs=8,
    on_bufs=4,
    onn_bufs=4,
    l_bufs=8,
    y_bufs=3,
    v_before_k=False,
    xb_bufs=2,
    outproj_fill=True,
    exact_diag_exp=False,
    pair_interleave=True,
    s_fp8=True,
    proj_fp8=1,
    v_late=True,
    k_first_dma=True,
    defer_pv=True,
    v_defer=False,
    v_defer0=False,
    act_tail_copy=True,
    exact_diag_last=False,
    split_last_outproj=False,
    compact_diag=True,
    v_copy_act_until=0,
    q_first_late=True,
    x8_persist=False,
    alt_tail_copy=True,
    tail_yp_in_s=True,
    tail_half_dma=False,
    v_fill=3,
    s_fill=1,
    pv_fill=0,
    norm_splits=1,
    k_split_fine=True,
    k_split_first1=True,
    wv_sp_half=False,
    qk_copy_act_it1=False,
    small_dma_order='tiw',
)


def build_program(opts=None):
    o = dict(DEFAULT_OPTS)
    if opts:
        o.update(opts)
    pv_stream = o["pv_stream"]
    # Legal dtype groups (HW verifier: matmul operands must share dtype when
    # fp32/f32r is involved):
    #   proj group (x, wq, wk, wv): bf16 (half DMA) or f32r
    #   attn group (qt, kt, P, v, onorm, wp, id): bf16 or f32r
    PROJ = BF if o["proj_bf16"] else F32R
    PROJ_D = BF if o["proj_bf16"] else F32
    ATTN = BF if o["attn_bf16"] else F32R
    ATTN_D = BF if o["attn_bf16"] else F32
    assert o["attn_bf16"] or not pv_stream, "pv_stream needs bf16 V"
    nc = bacc.Bacc("TRN2", target_bir_lowering=False, debug=False, num_devices=N_CORES)

    xt_d = nc.dram_tensor("xt", [E, T], PROJ_D, kind="ExternalInput").ap()
    x8_d = nc.dram_tensor("x8", [E, T], F8, kind="ExternalInput").ap()
    wqt_d = nc.dram_tensor("wqt", [E, GE],
                           F8 if o["proj_fp8"] >= 1 else PROJ_D, kind="ExternalInput").ap()
    wkt_d = nc.dram_tensor("wkt", [E, GE],
                           F8 if o["proj_fp8"] >= 2 else PROJ_D, kind="ExternalInput").ap()
    wvt_d = nc.dram_tensor("wvt", [E, GE], PROJ_D, kind="ExternalInput").ap()
    wpt_d = nc.dram_tensor("wpt", [GE, E], ATTN_D, kind="ExternalInput").ap()
    tri_d = nc.dram_tensor("tri", [KB, KB], ATTN_D, kind="ExternalInput").ap()
    id_d = nc.dram_tensor("ident", [128, 128], ATTN_D, kind="ExternalInput").ap()
    y_d = nc.dram_tensor("y", [T, E], BF, kind="ExternalOutput").ap()

    def slot(hb):
        return slice(hb * (D + 1), (hb + 1) * (D + 1))

    with tile.TileContext(nc) as tc:
        ctx = ExitStack()
        wpool = ctx.enter_context(tc.tile_pool(name="weights", bufs=1))
        qkpool = ctx.enter_context(tc.tile_pool(name="qk", bufs=1))
        vpool = ctx.enter_context(tc.tile_pool(name="vsb", bufs=1))
        xpool = ctx.enter_context(tc.tile_pool(name="xin", bufs=o["x_bufs"]))
        ppool = ctx.enter_context(tc.tile_pool(name="ptile", bufs=o["p_bufs"]))
        onpool = ctx.enter_context(tc.tile_pool(name="onorm", bufs=o["on_bufs"]))
        onnpool = ctx.enter_context(tc.tile_pool(name="on", bufs=o["onn_bufs"]))
        lpool = ctx.enter_context(tc.tile_pool(name="lbc", bufs=o["l_bufs"]))
        ypool = ctx.enter_context(tc.tile_pool(name="ystage", bufs=o["y_bufs"]))
        s_ps = ctx.enter_context(tc.tile_pool(name="s_ps", bufs=o["s_bufs"], space="PSUM"))
        o_ps = ctx.enter_context(tc.tile_pool(name="o_ps", bufs=o["o_bufs"], space="PSUM"))
        aux_ps = ctx.enter_context(tc.tile_pool(name="aux_ps", bufs=o["aux_bufs"], space="PSUM"))

        wq_sb = wpool.tile([128, KC, GE], F8 if o["proj_fp8"] >= 1 else PROJ)
        wk_sb = wpool.tile([128, KC, GE], F8 if o["proj_fp8"] >= 2 else PROJ)
        wv_sb = wpool.tile([128, KC, GE], PROJ)
        wp_sb = wpool.tile([128, 2, E], ATTN)
        tri_sb = wpool.tile([KB, KB], ATTN)
        id_sb = wpool.tile([128, 128], ATTN)

        if o["s_fp8"]:
            # d-half-split layout: partitions hb*32:(hb+1)*32 hold head hb,
            # dim1 is the d-half -- the DoubleRow reduction pair
            qt_sb = qkpool.tile([128, 2, T], F8)
            kt_sb = qkpool.tile([128, 2, T], F8)
        else:
            qt_sb = qkpool.tile([128, 2, T], ATTN)   # pair-stacked Q^T (moving)
            kt_sb = qkpool.tile([128, 2, T], ATTN)   # pair-stacked K^T (stationary)
        v_sb = vpool.tile([128, T // KB, GH * (D + 1)], ATTN)
        x8_all = None
        if o["proj_fp8"] and o["x8_persist"]:
            x8_all = qkpool.tile([128, KC, T], F8)

        if o["act_preload"]:
            # Tiny exp at t=0 so the 1.3us act-table load happens during the
            # startup DMA window, not before the first real softmax.
            warm = wpool.tile([128, 1], F32)
            nc.gpsimd.memset(warm[:], 0.0)
            nc.scalar.activation(out=warm[:], in_=warm[:], func=EXP)

        # ones columns of the augmented V (col D of each 65-wide head slot)
        v_ones = v_sb.rearrange("p b (h c) -> p (b h) c", c=D + 1)[:, :, D:D + 1]
        nc.gpsimd.memset(v_ones, 1.0)

        P_DT = ATTN

        def mload(eng, out_ap, in_ap, r=(PROJ == F32R)):
            eng.dma_start(out=out_ap, in_=in_ap.bitcast(F32R) if r else in_ap)

        def normalize(o_p, onorm, h, splits=1):
            w = QB // splits
            for s in range(splits):
                qs = slice(s * w, (s + 1) * w)
                strip = lpool.tile([1, w], F32, tag="strip", name="strip")
                nc.vector.reciprocal(out=strip[:], in_=o_p[D:D + 1, qs])
                lb = lpool.tile([D, w], F32, tag="lb", name="lb")
                nc.gpsimd.partition_broadcast(lb[:], strip[:])
                nc.vector.tensor_mul(onorm[h * D:(h + 1) * D, qs], o_p[0:D, qs], lb[:])

        def outproj_unit(yt, q0, onorms, qt, nh, stream_dma, act_copy=False,
                         use_s=False, half_dma=False):
            if use_s:
                yp = s_ps.tile([128, 512], F32, tag="s", name="yps")
            else:
                yp = aux_ps.tile([128, 512], F32, tag="aux", name="yp")
            for pair in range(2):
                nc.tensor.matmul(yp[:],
                                 onorms[pair][:, qt * KB:(qt + 1) * KB],
                                 wp_sb[:, pair, nh * 512:(nh + 1) * 512],
                                 start=(pair == 0), stop=(pair == 1))
            if act_copy:
                nc.scalar.copy(out=yt[:, qt, nh * 512:(nh + 1) * 512], in_=yp[:])
            else:
                nc.vector.tensor_copy(out=yt[:, qt, nh * 512:(nh + 1) * 512], in_=yp[:])
            if stream_dma and half_dma:
                nc.sync.dma_start(
                    out=y_d[q0 + qt * KB:q0 + (qt + 1) * KB, nh * 512:(nh + 1) * 512],
                    in_=yt[:, qt, nh * 512:(nh + 1) * 512])
            elif stream_dma and nh == 1:
                nc.sync.dma_start(out=y_d[q0 + qt * KB:q0 + (qt + 1) * KB, :],
                                  in_=yt[:, qt, :])

        def outproj_units(q0, onorms, stream_dma=False):
            yt = ypool.tile([128, QB // KB, E], BF, tag="y", name="yt")
            units = [(yt, q0, onorms, qt, nh, stream_dma)
                     for qt in range(QB // KB) for nh in range(2)]
            fin = []
            if not stream_dma:
                fin.append(lambda: nc.sync.dma_start(
                    out=y_d[q0:q0 + QB, :].rearrange("(a p) n -> p a n", p=128), in_=yt[:]))
            return units, fin

        def emit_outproj(q0, onorms, stream_dma=False):
            units, fin = outproj_units(q0, onorms, stream_dma)
            for u in units:
                outproj_unit(*u)
            for f in fin:
                f()

        prev_block = None  # (q0, onorms) awaiting out-projection
        pv_pending = None  # (qb, ptls, emit_pv) awaiting the P@V phase
        v_pending = None   # (tch, xts) awaiting the deferred V projection
        run_pv_phase = None
        for tch in range(NTCH):
            ts0 = tch * TCH
            # --- input DMAs (weights ride along with the first t-chunk) ---
            xb = xpool.tile([128, KC, TCH], PROJ, tag="xbig", name="xb", bufs=o["xb_bufs"])
            xr = xt_d.rearrange("(c p) t -> p c t", p=128)
            x8b = None
            if o["proj_fp8"]:
                x8r = x8_d.rearrange("(c p) t -> p c t", p=128)
                if o["x8_persist"]:
                    x8b = x8_all[:, :, ts0:ts0 + TCH]
                else:
                    x8b = xpool.tile([128, KC, TCH], F8, tag="x8big", name="x8b",
                                     bufs=o["xb_bufs"])
            if tch == 0:
                # halved transfers interleaved with weight halves so the first
                # projection matmuls start as soon as possible
                wqr = wqt_d.rearrange("(c p) n -> p c n", p=128)
                wkr = wkt_d.rearrange("(c p) n -> p c n", p=128)
                # (fp8 proj: wq/wk dram tensors already declared F8)
                wvr = wvt_d.rearrange("(c p) n -> p c n", p=128)
                if o["proj_fp8"]:
                    if o["k_first_dma"]:
                        # K path first (widest bf16 transfers gate the first S)
                        if o["k_split_fine"]:
                            if o["k_split_first1"]:
                                mload(nc.sync, xb[:, 0:1, :], xr[:, 0:1, ts0:ts0 + TCH])
                                mload(nc.sync, wk_sb[:, 0:1, :], wkr[:, 0:1, :])
                                mload(nc.sync, xb[:, 1:2, :], xr[:, 1:2, ts0:ts0 + TCH])
                                mload(nc.sync, wk_sb[:, 1:2, :], wkr[:, 1:2, :])
                            else:
                                mload(nc.sync, xb[:, 0:2, :], xr[:, 0:2, ts0:ts0 + TCH])
                                mload(nc.sync, wk_sb[:, 0:2, :], wkr[:, 0:2, :])
                            mload(nc.sync, xb[:, 2:4, :], xr[:, 2:4, ts0:ts0 + TCH])
                            mload(nc.sync, wk_sb[:, 2:4, :], wkr[:, 2:4, :])
                            mload(nc.sync, xb[:, 4:8, :], xr[:, 4:8, ts0:ts0 + TCH])
                            mload(nc.sync, wk_sb[:, 4:8, :], wkr[:, 4:8, :])
                        else:
                            mload(nc.sync, xb[:, 0:4, :], xr[:, 0:4, ts0:ts0 + TCH])
                            mload(nc.sync, wk_sb[:, 0:4, :], wkr[:, 0:4, :])
                            mload(nc.sync, xb[:, 4:8, :], xr[:, 4:8, ts0:ts0 + TCH])
                            mload(nc.sync, wk_sb[:, 4:8, :], wkr[:, 4:8, :])
                        nc.sync.dma_start(out=x8b[:, 0:4, :], in_=x8r[:, 0:4, ts0:ts0 + TCH])
                        nc.sync.dma_start(out=wq_sb[:], in_=wqr[:])
                        nc.sync.dma_start(out=x8b[:, 4:8, :], in_=x8r[:, 4:8, ts0:ts0 + TCH])
                        if o["x8_persist"]:
                            for lt in range(1, NTCH):
                                nc.sync.dma_start(
                                    out=x8_all[:, :, lt * TCH:(lt + 1) * TCH],
                                    in_=x8r[:, :, lt * TCH:(lt + 1) * TCH])
                    else:
                        nc.sync.dma_start(out=x8b[:, 0:4, :], in_=x8r[:, 0:4, ts0:ts0 + TCH])
                        nc.sync.dma_start(out=wq_sb[:], in_=wqr[:])
                        nc.sync.dma_start(out=x8b[:, 4:8, :], in_=x8r[:, 4:8, ts0:ts0 + TCH])
                        mload(nc.sync, xb[:, 0:4, :], xr[:, 0:4, ts0:ts0 + TCH])
                        mload(nc.sync, wk_sb[:, 0:4, :], wkr[:, 0:4, :])
                        mload(nc.sync, xb[:, 4:8, :], xr[:, 4:8, ts0:ts0 + TCH])
                        mload(nc.sync, wk_sb[:, 4:8, :], wkr[:, 4:8, :])
                else:
                    mload(nc.sync, xb[:, 0:2, :], xr[:, 0:2, ts0:ts0 + TCH])
                    mload(nc.sync, wq_sb[:, 0:2, :], wqr[:, 0:2, :])
                    mload(nc.sync, xb[:, 2:4, :], xr[:, 2:4, ts0:ts0 + TCH])
                    mload(nc.sync, wq_sb[:, 2:4, :], wqr[:, 2:4, :])
                    mload(nc.sync, xb[:, 4:8, :], xr[:, 4:8, ts0:ts0 + TCH])
                    mload(nc.sync, wq_sb[:, 4:8, :], wqr[:, 4:8, :])
                    mload(nc.sync, wk_sb[:, 0:4, :], wkr[:, 0:4, :])
                    mload(nc.sync, wk_sb[:, 4:8, :], wkr[:, 4:8, :])
                weng = nc.gpsimd if o["dma_spread"] else nc.sync
                mload(nc.sync if o["wv_sp_half"] else weng, wv_sb[:, 0:4, :], wvr[:, 0:4, :])
                mload(weng, wv_sb[:, 4:8, :], wvr[:, 4:8, :])
                aload = (ATTN == F32R)
                small = {
                    't': lambda: mload(nc.sync, tri_sb[:], tri_d, r=aload),
                    'i': lambda: mload(nc.sync, id_sb[:], id_d, r=aload),
                    'w': lambda: mload(nc.sync, wp_sb[:],
                                       wpt_d.rearrange("(c p) n -> p c n", p=128), r=aload),
                }
                for c in o["small_dma_order"]:
                    small[c]()
            else:
                if o["proj_fp8"] and not o["x8_persist"]:
                    nc.sync.dma_start(out=x8b[:], in_=x8r[:, :, ts0:ts0 + TCH])
                mload(nc.sync, xb[:], xr[:, :, ts0:ts0 + TCH])
            xts = [xb[:, kc, :] for kc in range(KC)]

            # --- Q/K/V projections for this t-chunk ---
            def proj_qk(which, w_sb, t_sb, fp8):
                for pair in range(2):
                    psl = slice(pair * 128, (pair + 1) * 128)
                    pp = aux_ps.tile([128, TCH], F32, tag="aux", name=which)
                    if fp8:
                        DRm = mybir.MatmulPerfMode.DoubleRow
                        for kcp in range(0, KC, 2):
                            nc.tensor.matmul(pp[:], w_sb[:, kcp:kcp + 2, psl],
                                             x8b[:, kcp:kcp + 2, :],
                                             start=(kcp == 0), stop=(kcp == KC - 2),
                                             perf_mode=DRm, tile_position=(0, 0))
                    else:
                        for kc in range(KC):
                            nc.tensor.matmul(pp[:], w_sb[:, kc, psl], xts[kc][:],
                                             start=(kc == 0), stop=(kc == KC - 1))
                    # pair-stacked (bf16 S) or d-half-split fp8 (DoubleRow S):
                    # the host reorders W columns so slot `pair` is the d-half.
                    # Iteration 1's copies ride the act engine, idle between
                    # block 0's and block 1's exp streams.
                    if tch == 1 and o["qk_copy_act_it1"]:
                        nc.scalar.copy(out=t_sb[:, pair, ts0:ts0 + TCH], in_=pp[:])
                    else:
                        nc.vector.tensor_copy(out=t_sb[:, pair, ts0:ts0 + TCH], in_=pp[:])

            def proj_v(vtch=tch, vxts=None):
                vxts = vxts if vxts is not None else xts
                for tsub in range(TCH // KB):
                    vp = aux_ps.tile([128, GE], F32, tag="aux", name="vp")
                    for kc in range(KC):
                        nc.tensor.matmul(vp[:], vxts[kc][:, tsub * KB:(tsub + 1) * KB],
                                         wv_sb[:, kc, :],
                                         start=(kc == 0), stop=(kc == KC - 1))
                    tb = vtch * (TCH // KB) + tsub
                    vdst = v_sb[:, tb, :].rearrange("p (h c) -> p h c", c=D + 1)[:, :, 0:D]
                    vsrc = vp.rearrange("p (h c) -> p h c", c=D)
                    if vtch < o["v_copy_act_until"]:
                        nc.scalar.copy(out=vdst, in_=vsrc)
                    else:
                        nc.vector.tensor_copy(out=vdst, in_=vsrc)

            if o["v_late"]:
                # V is emitted mid-attention (after the S phase) so the
                # activation engine gets its exp stream sooner
                if tch >= 1 and o["q_first_late"]:
                    proj_qk("qp", wq_sb, qt_sb, o["proj_fp8"] >= 1)
                    proj_qk("kp", wk_sb, kt_sb, o["proj_fp8"] >= 2)
                else:
                    proj_qk("kp", wk_sb, kt_sb, o["proj_fp8"] >= 2)
                    proj_qk("qp", wq_sb, qt_sb, o["proj_fp8"] >= 1)
            else:
                proj_qk("qp", wq_sb, qt_sb, o["proj_fp8"] >= 1)
                if o["v_before_k"]:
                    proj_v()
                    proj_qk("kp", wk_sb, kt_sb, o["proj_fp8"] >= 2)
                else:
                    proj_qk("kp", wk_sb, kt_sb, o["proj_fp8"] >= 2)
                    proj_v()

            # deferred out-projection of the previous q-block: either emitted
            # here (proj matmuls cover its normalize latency) or spread into
            # the attention stream as stall fillers
            fill_units, fill_fin = [], []
            if o["defer_outproj"] and prev_block is not None:
                if o["outproj_fill"]:
                    fill_units, fill_fin = outproj_units(*prev_block)
                    fill_units = list(fill_units)
                else:
                    emit_outproj(*prev_block)
                prev_block = None

            def fill(n=1):
                for _ in range(n):
                    if fill_units:
                        outproj_unit(*fill_units.pop(0))

            # --- attention: S for q-block qb == tch; P@V optionally deferred
            # one iteration so the next block's S feeds the act engine first ---
            qb = tch
            q0 = qb * QB
            nk = (q0 + QB) // KB
            nfull = nk - 4
            onorms = []

            DR = mybir.MatmulPerfMode.DoubleRow

            def s_matmul(out_ap, pair, h, jsl, qsl):
                if o["s_fp8"]:
                    hb32 = (pair * 2 + h) * 32
                    hsl = slice(hb32, hb32 + 32)
                    nc.tensor.matmul(out_ap, kt_sb[hsl, :, jsl], qt_sb[hsl, :, qsl],
                                     start=True, stop=True, perf_mode=DR,
                                     tile_position=(hb32, 0))
                else:
                    bsl = slice(h * D, (h + 1) * D)
                    nc.tensor.matmul(out_ap, kt_sb[bsl, pair, jsl], qt_sb[bsl, pair, qsl],
                                     start=True, stop=True)

            def emit_s(pair, h, ptl, q0=q0, nk=nk, nfull=nfull):
                    for j2 in range(0, nfull, 2):
                        sp = s_ps.tile([128, 2, QB], F32, tag="s", name="sp")
                        for jj in range(2):
                            j = j2 + jj
                            s_matmul(sp[:, jj, :], pair, h,
                                     slice(j * KB, (j + 1) * KB), slice(q0, q0 + QB))
                        pt = ppool.tile([128, 2, QB], P_DT, tag="p", name="pt")
                        nc.scalar.activation(out=pt.rearrange("p a b -> p (a b)"),
                                             in_=sp.rearrange("p a b -> p (a b)"),
                                             func=EXP, scale=SCALE)
                        ptl.append(pt)
                    for j2 in range(nfull, nk, 2):
                        r0 = (j2 - nfull) * KB
                        r1 = r0 + KB
                        sp = s_ps.tile([128, 2, QB], F32, tag="s", name="sp")
                        s_matmul(sp[:, 0, r0:QB], pair, h,
                                 slice(j2 * KB, (j2 + 1) * KB), slice(q0 + r0, q0 + QB))
                        pt = ppool.tile([128, 2, QB], P_DT, tag="p", name="pt")
                        if o["compact_diag"]:
                            # second block written left-shifted so one exp
                            # covers exactly the causal columns of both blocks
                            s_matmul(sp[:, 1, 0:QB - r1], pair, h,
                                     slice((j2 + 1) * KB, (j2 + 2) * KB),
                                     slice(q0 + r1, q0 + QB))
                            nc.scalar.activation(
                                out=pt.rearrange("p a b -> p (a b)")[:, r0:2 * QB - r1],
                                in_=sp.rearrange("p a b -> p (a b)")[:, r0:2 * QB - r1],
                                func=EXP, scale=SCALE)
                            nc.gpsimd.tensor_mul(pt[:, 0, r0:r0 + KB],
                                                 pt[:, 0, r0:r0 + KB], tri_sb[:])
                            nc.gpsimd.tensor_mul(pt[:, 1, 0:KB],
                                                 pt[:, 1, 0:KB], tri_sb[:])
                            ptl.append(pt)
                            continue
                        s_matmul(sp[:, 1, r1:QB], pair, h,
                                 slice((j2 + 1) * KB, (j2 + 2) * KB), slice(q0 + r1, q0 + QB))
                        if o["exact_diag_exp"] or (o["exact_diag_last"] and qb == NQB - 1):
                            nc.scalar.activation(out=pt[:, 0, r0:QB], in_=sp[:, 0, r0:QB],
                                                 func=EXP, scale=SCALE)
                            nc.scalar.activation(out=pt[:, 1, r1:QB], in_=sp[:, 1, r1:QB],
                                                 func=EXP, scale=SCALE)
                        else:
                            nc.scalar.activation(
                                out=pt.rearrange("p a b -> p (a b)")[:, r0:2 * QB],
                                in_=sp.rearrange("p a b -> p (a b)")[:, r0:2 * QB],
                                func=EXP, scale=SCALE)
                        nc.gpsimd.tensor_mul(pt[:, 0, r0:r0 + KB], pt[:, 0, r0:r0 + KB], tri_sb[:])
                        nc.gpsimd.tensor_mul(pt[:, 1, r1:r1 + KB], pt[:, 1, r1:r1 + KB], tri_sb[:])
                        ptl.append(pt)

            def emit_pv(pair, h, ptl, onorm, pqb=qb, pnk=nk, pnfull=nfull):
                    o_p = o_ps.tile([D + 1, QB], F32, tag="o", name="o_t")
                    for j in range(pnfull):
                        j2, jj = divmod(j, 2)
                        nc.tensor.matmul(o_p[:],
                                         v_sb[:, j, slot(pair * 2 + h)],
                                         ptl[j2][:, jj, :],
                                         start=(j == 0), stop=False)
                    fill(1)
                    for j in range(pnfull, pnk):
                        j2, jj = divmod(j, 2)
                        r = (j - pnfull) * KB
                        c0 = 0 if (jj == 1 and o["compact_diag"]) else r
                        nc.tensor.matmul(o_p[:, r:QB],
                                         v_sb[:, j, slot(pair * 2 + h)],
                                         ptl[j2][:, jj, c0:c0 + QB - r],
                                         start=(j == 0 if pnfull == 0 else False),
                                         stop=(j == pnk - 1))
                    normalize(o_p, onorm, h,
                              splits=(o["norm_splits_last"] if pqb == NQB - 1
                                      else o["norm_splits"]))

            def run_pv_phase(pend):
                pqb, pptls, ppv = pend
                po = [onpool.tile([128, QB], ATTN, tag="onorm", name="onorm_t")
                      for _ in range(2)]
                for pair in range(2):
                    for h in range(2):
                        ppv(pair, h, pptls[(pair, h)], po[pair])
                    fill(o["pv_fill"])
                return (pqb * QB, po)

            assert not pv_stream
            if o["pair_interleave"]:
                ptls = {}
                for pair in range(2):
                    for h in range(2):
                        ptls[(pair, h)] = []
                        emit_s(pair, h, ptls[(pair, h)])
                    fill(o["s_fill"])
                if o["v_late"]:
                    if o["v_defer"]:
                        # hybrid deferral: V(t) runs after S(t+1) so each
                        # iteration's S feeds the act engine first; the last
                        # chunk's V stays inline to keep the epilogue short
                        if v_pending is not None:
                            proj_v(*v_pending)
                        if tch == NTCH - 1:
                            proj_v()
                    elif o["v_defer0"]:
                        # defer only V(0): iteration 1 reaches S(1) sooner,
                        # closing the act-engine gap at the qb0->qb1 boundary
                        if v_pending is not None:
                            proj_v(*v_pending)
                        if tch != 0:
                            proj_v()
                    else:
                        proj_v()
                    fill(o["v_fill"])
                if o["defer_pv"]:
                    if pv_pending is not None:
                        prev_block = run_pv_phase(pv_pending)
                    pv_pending = (qb, ptls, emit_pv)
                    if o["v_defer"]:
                        v_pending = (tch, xts) if tch < NTCH - 1 else None
                    elif o["v_defer0"]:
                        v_pending = (tch, xts) if tch == 0 else None
                    for u in fill_units:
                        outproj_unit(*u)
                    for f in fill_fin:
                        f()
                    continue
                onorms = [onpool.tile([128, QB], ATTN, tag="onorm", name="onorm_t")
                          for _ in range(2)]
                for pair in range(2):
                    for h in range(2):
                        emit_pv(pair, h, ptls[(pair, h)], onorms[pair])
                    fill(1)
            else:
                for pair in range(2):
                    onorm = onpool.tile([128, QB], ATTN, tag="onorm", name="onorm_t")
                    ptls = [[], []]
                    for h in range(2):
                        emit_s(pair, h, ptls[h])
                    fill(2)
                    for h in range(2):
                        emit_pv(pair, h, ptls[h], onorm)
                    onorms.append(onorm)


            for u in fill_units:
                outproj_unit(*u)
            for f in fill_fin:
                f()

            # --- output projection: either inline or deferred to the next
            # iteration (after its projections) ---
            if not o["defer_outproj"]:
                emit_outproj(q0, onorms)
            else:
                prev_block = (q0, onorms)

        if o.get("defer_pv") and pv_pending is not None:
            # epilogue: deferred V, then P@V of the last q-block, with the
            # prior block's out-projection as fillers
            if o["v_defer"] and v_pending is not None:
                proj_v(*v_pending)
            fill_units, fill_fin = [], []
            if prev_block is not None:
                fill_units, fill_fin = outproj_units(*prev_block)
                fill_units = list(fill_units)
            prev_block = run_pv_phase(pv_pending)
            for u in fill_units:
                outproj_unit(*u, act_copy=o["act_tail_copy"])
            for f in fill_fin:
                f()

        if o["defer_outproj"] and prev_block is not None:
            q0f, onormsf = prev_block
            ytf = ypool.tile([128, QB // KB, E], BF, tag="y", name="ytf")
            if o["split_last_outproj"]:
                # quarter-contraction units, 2-deep pipelined: the first three
                # 64-partition matmuls of each unit only need the earlier
                # normalize chains, covering the last head's normalize latency
                units = [(qt, nh) for qt in range(QB // KB) for nh in range(2)]
                quarters = [(0, 0), (0, 1), (1, 0), (1, 1)]

                def u_mms(yp, qt, nh, qrange):
                    for k in qrange:
                        pair, hh = quarters[k]
                        hsl = slice(hh * 64, (hh + 1) * 64)
                        nc.tensor.matmul(yp[:],
                                         onormsf[pair][hsl, qt * KB:(qt + 1) * KB],
                                         wp_sb[hsl, pair, nh * 512:(nh + 1) * 512],
                                         start=(k == 0), stop=(k == 3),
                                         tile_position=(hh * 64, 0))

                def u_done(yp, qt, nh):
                    u_mms(yp, qt, nh, [3])
                    eng = nc.scalar if o["act_tail_copy"] else nc.vector
                    (eng.copy if o["act_tail_copy"] else eng.tensor_copy)(
                        out=ytf[:, qt, nh * 512:(nh + 1) * 512], in_=yp[:])
                    if nh == 1:
                        nc.sync.dma_start(
                            out=y_d[q0f + qt * KB:q0f + (qt + 1) * KB, :],
                            in_=ytf[:, qt, :])

                pend = []  # (yp, qt, nh) with quarters 0-2 issued
                for qt, nh in units:
                    yp = aux_ps.tile([128, 512], F32, tag="aux", name="ypf")
                    u_mms(yp, qt, nh, [0, 1, 2])
                    pend.append((yp, qt, nh))
                    if len(pend) == 2:
                        u_done(*pend.pop(0))
                while pend:
                    u_done(*pend.pop(0))
            else:
                for qt in range(QB // KB):
                    for nh in range(2):
                        ac = o["act_tail_copy"] and (nh == 0 if o["alt_tail_copy"]
                                                     else True)
                        outproj_unit(ytf, q0f, onormsf, qt, nh, True, act_copy=ac,
                                     use_s=(o["tail_yp_in_s"] and nh == 1),
                                     half_dma=o["tail_half_dma"])

        ctx.close()

    nc.compile()
    return nc


_NC = {}


def _get_program(opts=None):
    key = tuple(sorted((opts or {}).items()))
    if key not in _NC:
        _NC[key] = build_program(opts)
    return _NC[key]


def _make_in_maps(x, Wq, Wk, Wv, Wp, opts=None):
    o = dict(DEFAULT_OPTS)
    if opts:
        o.update(opts)
    pdt = BF_NP if o["proj_bf16"] else np.float32
    adt = BF_NP if o["attn_bf16"] else np.float32
    x = np.asarray(x, dtype=np.float32)
    wqt = np.asarray(Wq, np.float32).T
    wkt = np.asarray(Wk, np.float32).T
    if o["s_fp8"]:
        # reorder per-core GE columns to the d-half-split layout:
        # half-major, then head, then d-within-half
        perm = np.array([hb * 64 + half * 32 + d
                         for half in range(2) for hb in range(4) for d in range(32)])
    else:
        perm = np.arange(GE)
    wvt = np.asarray(Wv, np.float32).T
    wpt = np.asarray(Wp, np.float32).T
    tri = (np.arange(KB)[:, None] <= np.arange(KB)[None, :]).astype(adt)
    ident = np.eye(128, dtype=adt)
    in_maps = []
    for c in range(N_CORES):
        b, hg = c // 4, c % 4
        f8np = ml_dtypes.float8_e4m3
        in_maps.append({
            "xt": np.ascontiguousarray(x[b].T).astype(pdt),
            "x8": np.ascontiguousarray(x[b].T).astype(f8np),
            "wqt": np.ascontiguousarray(wqt[:, hg * GE:(hg + 1) * GE][:, perm]).astype(
                f8np if o["proj_fp8"] >= 1 else pdt),
            "wkt": np.ascontiguousarray(wkt[:, hg * GE:(hg + 1) * GE][:, perm]).astype(
                f8np if o["proj_fp8"] >= 2 else pdt),
            "wvt": np.ascontiguousarray(wvt[:, hg * GE:(hg + 1) * GE]).astype(pdt),
            "wpt": np.ascontiguousarray(wpt[hg * GE:(hg + 1) * GE, :]).astype(adt),
            "tri": tri,
            "ident": ident,
        })
    return in_maps


def run_cores(x, Wq, Wk, Wv, Wp, bp, **spmd_kwargs):
    """Run the 8-core program; returns (y_full, BassKernelResults)."""
    nc = _get_program()
    in_maps = _make_in_maps(x, Wq, Wk, Wv, Wp)
    res = run_bass_kernel_spmd(nc, in_maps, list(range(N_CORES)), **spmd_kwargs)
    parts = [res.results[c]["y"] for c in range(N_CORES)]
    y = np.empty((B, T, E), np.float32)
    for b in range(B):
        acc = parts[4 * b].astype(np.float32)
        for hg in range(1, 4):
            acc = acc + parts[4 * b + hg].astype(np.float32)
        y[b] = acc
    y += np.asarray(bp, np.float32)[None, None, :]
    return y, res


def kernel(x, Wq, Wk, Wv, Wp, bp):
    y, _ = run_cores(x, Wq, Wk, Wv, Wp, bp)
    return y


def bench(x, Wq, Wk, Wv, Wp, bp, iters=12):
    """Time repeated on-device executions of the compiled program.

    Returns (y_full, list_of_call_seconds). Builds the sharded jit once;
    inputs are device-resident; fresh donated zero outputs are staged
    outside the timed region each iteration.
    """
    import time

    import jax
    import numpy as np_
    from jax.experimental.shard_map import shard_map
    from jax.sharding import Mesh, NamedSharding, PartitionSpec

    from concourse import bass2jax, mybir as mb

    nc = _get_program()
    in_maps = _make_in_maps(x, Wq, Wk, Wv, Wp)
    n_cores = N_CORES
    bass2jax.install_neuronx_cc_hook()

    partition_name = nc.partition_id_tensor.name if nc.partition_id_tensor else None
    in_names, out_names, out_avals, zero_outs = [], [], [], []
    for alloc in nc.m.functions[0].allocations:
        if not isinstance(alloc, mb.MemoryLocationSet):
            continue
        name = alloc.memorylocations[0].name
        if alloc.kind == "ExternalInput":
            if name != partition_name:
                in_names.append(name)
        elif alloc.kind == "ExternalOutput":
            out_names.append(name)
            shape = tuple(alloc.tensor_shape)
            dtype = mb.dt.np(alloc.dtype)
            out_avals.append(jax.core.ShapedArray(shape, dtype))
            zero_outs.append(np_.zeros(shape, dtype))
    n_params = len(in_names)
    all_in_names = in_names + out_names
    if partition_name is not None:
        all_in_names = all_in_names + [partition_name]

    def _body(*args):
        operands = list(args)
        if partition_name is not None:
            operands.append(bass2jax.partition_id_tensor())
        outs = bass2jax._bass_exec_p.bind(
            *operands,
            out_avals=tuple(out_avals),
            in_names=tuple(all_in_names),
            out_names=tuple(out_names),
            lowering_input_output_aliases=(),
            sim_require_finite=True,
            sim_require_nnan=True,
            nc=nc,
        )
        return tuple(outs)

    devices = jax.devices()[:n_cores]
    mesh = Mesh(np_.asarray(devices), ("core",))
    donate = tuple(range(n_params, n_params + len(out_names)))
    sharded = jax.jit(
        shard_map(_body, mesh=mesh,
                  in_specs=(PartitionSpec("core"),) * (n_params + len(out_names)),
                  out_specs=(PartitionSpec("core"),) * len(out_names),
                  check_rep=False),
        donate_argnums=donate, keep_unused=True,
    )
    sh = NamedSharding(mesh, PartitionSpec("core"))
    concat_in = [
        jax.device_put(
            np_.concatenate([np_.asarray(in_maps[c][nm]) for c in range(n_cores)], axis=0), sh)
        for nm in in_names
    ]
    zeros_np = [np_.zeros((n_cores * z.shape[0], *z.shape[1:]), z.dtype) for z in zero_outs]

    times = []
    out_arrs = None
    for it in range(iters):
        dz = [jax.device_put(z, sh) for z in zeros_np]
        jax.block_until_ready(dz)
        t0 = time.perf_counter()
        out_arrs = sharded(*concat_in, *dz)
        jax.block_until_ready(out_arrs)
        times.append(time.perf_counter() - t0)

    parts = [
        np_.asarray(out_arrs[i]).reshape(n_cores, *out_avals[i].shape)
        for i, nm in enumerate(out_names)
    ]
    yi = out_names.index("y")
    y = np_.empty((B, T, E), np_.float32)
    for b in range(B):
        acc = parts[yi][4 * b].astype(np_.float32)
        for hg in range(1, 4):
            acc = acc + parts[yi][4 * b + hg].astype(np_.float32)
        y[b] = acc
    y += np_.asarray(bp, np_.float32)[None, None, :]
    return y, times
